# revision 4
# baseline (speedup 1.0000x reference)
"""AttentiveConv3d (sparse_attention) Trainium2 kernel v2 — self-contained.

kernel(**inputs) takes FULL inputs
    x [2,128,16,28,28] f32, q [2,1,64] f32, W_out [128,128] f32, b_out [128] f32
and returns the FULL output [2,128,16,28,28] f32.

Sharding: 8 cores = 2 batches x 4 T-chunks; core i handles batch i//4, output
frames 4*(i%4)..4*(i%4)+3 from 6 input frames (1-frame halo, host-sliced).

Math (equivalent to the reference):
    z   = qm^T @ x      (broadcast logit field, per-channel head)
    E   = exp(z);  F = E * x            (bf16 fields, pads E=1 / F=0)
    num = Box3x3x3(F)   (separable: T pair-trick, W, H passes)
    den = Box3x3x3(E)   (dy-folded 24-row packs, PE matmul contraction, 2 halves)
    y   = W_out @ (num * (1/den))       (bias added host-side)

Engines: PE matmuls (z, den, r-broadcast, y-proj) in bf16; ACT exp +
reciprocal + y evac; DVE bulk elementwise (bf16 2x mode) incl the T/W/H box
passes and merged muls; Pool takes two F-muls and P12; cheap SBUF->SBUF DMAs
build the dy-folded den packs. All PSUM matmul targets are 512-col
bank-aligned (accumulation groups must not straddle a 2KB PSUM bank).
"""
from contextlib import ExitStack

import ml_dtypes
import numpy as np

import concourse.bass as bass
import concourse.tile as tile
from concourse import bacc, mybir
from concourse import bass_utils

F32 = mybir.dt.float32
F32R = mybir.dt.float32r
BF16 = mybir.dt.bfloat16
AF = mybir.ActivationFunctionType
ALU = mybir.AluOpType

C = 128
TIN, TOUT = 6, 4
HP, WP = 30, 30
HO, WO = 28, 28
NF = HP * WP        # 900
NOF = HO * WO       # 784
BLOB_W = 912  # qm | idm | wt | selA | selB | selr (all bf16)


def _build_nc(num_devices=8, reps=1, n_warm=2):
    nc = bacc.Bacc("TRN2", target_bir_lowering=False, debug=False,
                   num_devices=num_devices)
    d_xp = nc.dram_tensor("xp", [C, TIN, NOF], BF16, kind="ExternalInput").ap()
    d_blob = nc.dram_tensor("blob", [C, BLOB_W], BF16, kind="ExternalInput").ap()
    d_y = nc.dram_tensor("y", [C, TOUT, NOF], BF16, kind="ExternalOutput").ap()

    with tile.TileContext(nc) as tc:
        with ExitStack() as ctx:
            consts = ctx.enter_context(tc.tile_pool(name="consts", bufs=1))
            sb_x = ctx.enter_context(tc.tile_pool(name="sb_x", bufs=1))
            sb_e = ctx.enter_context(tc.tile_pool(name="sb_e", bufs=1))
            sb_f = ctx.enter_context(tc.tile_pool(name="sb_f", bufs=1))
            sb_p = ctx.enter_context(tc.tile_pool(name="sb_p", bufs=1))
            sb_s = ctx.enter_context(tc.tile_pool(name="sb_s", bufs=1))
            sb_g = ctx.enter_context(tc.tile_pool(name="sb_g", bufs=1))
            sb_n = ctx.enter_context(tc.tile_pool(name="sb_n", bufs=4))
            sb_m = ctx.enter_context(tc.tile_pool(name="sb_m", bufs=4))
            sb_y = ctx.enter_context(tc.tile_pool(name="sb_y", bufs=4))
            sb_r = ctx.enter_context(tc.tile_pool(name="sb_r", bufs=1))
            sb_pk = ctx.enter_context(tc.tile_pool(name="sb_pk", bufs=2))
            ps_big = ctx.enter_context(tc.tile_pool(name="ps_big", bufs=2, space="PSUM"))
            ps_rp = ctx.enter_context(tc.tile_pool(name="ps_rp", bufs=2, space="PSUM"))
            ps_d8 = ctx.enter_context(tc.tile_pool(name="ps_d8", bufs=2, space="PSUM"))
            ps_num = ctx.enter_context(tc.tile_pool(name="ps_num", bufs=1, space="PSUM"))

            blob_t = consts.tile([C, BLOB_W], BF16)
            nc.sync.dma_start(out=blob_t[:, 0:128], in_=d_blob[:, 0:128])
            qm = blob_t[:, 0:128]
            idm = blob_t[:, 128:256]
            wt = blob_t[:, 256:384]
            selA = blob_t[0:24, 384:392]
            selB = blob_t[0:24, 392:400]
            selr = blob_t[0:8, 400:912]   # [8, 512] = 4 x [8,128] bf16

            dummy = consts.tile([1, 4], F32, name="dummy")
            nc.vector.memset(dummy[:], 1.0)
            dummy2 = consts.tile([1, 4], BF16, name="dummy2")
            nc.scalar.add_instruction(
                mybir.InstActivation(
                    name=nc.get_next_instruction_name(),
                    func=mybir.ActivationFunctionType.Reciprocal,
                    ins=[nc.scalar.lower_ap(dummy[:]),
                         mybir.ImmediateValue(dtype=F32, value=0.0),
                         mybir.ImmediateValue(dtype=F32, value=1.0),
                         mybir.ImmediateValue(dtype=F32, value=0.0)],
                    outs=[nc.scalar.lower_ap(dummy2[:])],
                ))
            qmb = blob_t[:, 0:128]
            qrep = bass.AP(tensor=qmb.tensor, offset=qmb.offset,
                           ap=[list(qmb.ap[0]), [0, 3], [1, 128]])
            for i in range(n_warm):
                wrm = ps_big.tile([C, 1024], F32, tag="big", name=f"warm{i}")
                nc.tensor.matmul(wrm[:, 0:384], qm, qrep, start=True, stop=True)

            for _ in range(reps):
                _body(tc, nc, (d_xp, d_blob, blob_t), d_y, qm, idm, wt, selA, selB, selr,
                      sb_x, sb_e, sb_f, sb_p, sb_s, sb_g, sb_n, sb_m, sb_y,
                      sb_r, sb_pk, ps_big, ps_rp, ps_d8, ps_num)
    nc.compile()
    return nc


def _body(tc, nc, d_xp, d_y, qm, idm, wt, selA, selB, selr,
          sb_x, sb_e, sb_f, sb_p, sb_s, sb_g, sb_n, sb_m, sb_y,
          sb_r, sb_pk, ps_big, ps_rp, ps_d8, ps_num):
    d_xp, d_blob, blob_t = d_xp
    qrep_b = bass.AP(tensor=qm.tensor, offset=qm.offset,
                     ap=[list(qm.ap[0]), [0, 4], [1, 128]])
    # ---- SBUF field tiles -------------------------------------------------
    x128 = sb_x.tile([C, TIN * NOF], BF16, tag="x")
    e128 = sb_e.tile([C, TIN * NF], BF16, tag="e")
    f128 = sb_f.tile([C, TIN * NF], BF16, tag="f")
    ptile = sb_p.tile([C, 2 * NF], BF16, tag="p")       # P12 | P34
    s128 = sb_s.tile([C, TOUT * NF], BF16, tag="s")
    g128 = sb_g.tile([C, TOUT * 840], BF16, tag="g")    # [30,28] per frame

    # ---- pad init (Pool memsets; no data deps) ---------------------------
    for base, val in ((e128[:], 1.0), (f128[:], 0.0)):
        rows = bass.AP(tensor=base.tensor, offset=base.offset,
                       ap=[list(base.ap[0]), [NF, TIN], [870, 2], [1, WP]])
        nc.gpsimd.memset(rows, val)
        cols = bass.AP(tensor=base.tensor, offset=base.offset + WP,
                       ap=[list(base.ap[0]), [NF, TIN], [WP, HP - 2], [WP - 1, 2]])
        nc.gpsimd.memset(cols, val)

    # ---- input DMAs: single frames in order; deferred consts after ------
    for f in range(TIN):
        nc.sync.dma_start(out=x128[:, f * NOF:(f + 1) * NOF], in_=d_xp[:, f])
    nc.sync.dma_start(out=blob_t[:, 128:BLOB_W], in_=d_blob[:, 128:BLOB_W])

    # ---- phase A: z matmul (PE) + exp (ACT) ------------------------------
    def phase_a(f):
        zp = ps_big.tile([C, 1024], F32, tag="big", name=f"zp{f}")
        xf = x128[:, f * NOF:(f + 1) * NOF]
        nc.tensor.matmul(zp[:, 0:392], qm, xf[:, 0:392], start=True, stop=True)
        nc.tensor.matmul(zp[:, 512:904], qm, xf[:, 392:784], start=True, stop=True)
        eb = e128[:]
        ev = bass.AP(tensor=eb.tensor, offset=eb.offset + f * NF + 31,
                     ap=[list(eb.ap[0]), [14 * WP, 2], [WP, 14], [1, 28]])
        zv = (zp[:].rearrange("p (b k) -> p b k", b=2)[:, :, 0:392]
              .rearrange("p b (y x) -> p b y x", x=28))
        nc.scalar.activation(ev, zv, AF.Exp)

    def f_mul(f0, engine, nfr=2):
        if nfr == 1:
            ev = (e128[:, f0 * NF:(f0 + 1) * NF]
                  .rearrange("p (y x) -> p y x", y=HP)[:, 1:29, 1:29])
            fv = (f128[:, f0 * NF:(f0 + 1) * NF]
                  .rearrange("p (y x) -> p y x", y=HP)[:, 1:29, 1:29])
            xv = (x128[:, f0 * NOF:(f0 + 1) * NOF]
                  .rearrange("p (y x) -> p y x", y=HO))
        else:
            ev = (e128[:, f0 * NF:(f0 + 2) * NF]
                  .rearrange("p (t y x) -> p t y x", t=2, y=HP)[:, :, 1:29, 1:29])
            fv = (f128[:, f0 * NF:(f0 + 2) * NF]
                  .rearrange("p (t y x) -> p t y x", t=2, y=HP)[:, :, 1:29, 1:29])
            xv = (x128[:, f0 * NOF:(f0 + 2) * NOF]
                  .rearrange("p (t y x) -> p t y x", t=2, y=HO))
        eng = nc.gpsimd if engine == "pool" else nc.vector
        eng.tensor_mul(fv, ev, xv)

    # ---- den halves ------------------------------------------------------
    def mk_pack(name):
        return sb_pk.tile([24, NF], BF16, tag="pk", name=f"pk{name}")

    def pack_all(pk, fbase):
        # pack rows: 12h + 4dy + dt' (h slowest, then dy, then frame);
        # one DMA: all dy rows only need cols < 840 (den rhs max col 839)
        e2v = e128[0:2, :]
        for dy, eng in ((0, nc.sync), (1, nc.sync), (2, nc.sync)):
            src = bass.AP(tensor=e2v.tensor,
                          offset=e2v.offset + fbase * NF + WP * dy,
                          ap=[list(e2v.ap[0]), [NF, 4], [1, 840]])
            eng.dma_start(out=pk[8 * dy:8 * dy + 8, 0:840], in_=src)

    def den_half(name, pk, sel):
        pkv = pk[:].rearrange("p (y x) -> p y x", y=HP)
        r8 = sb_r.tile([8, 784], BF16, tag=f"r8{name}", name=f"r8{name}")
        for ch in range(2):
            d8p = ps_d8.tile([8, 392], F32, tag="d8", name=f"d8{name}{ch}")
            for dx in range(3):
                nc.tensor.matmul(
                    d8p[:], sel,
                    pkv[:, 14 * ch:14 * ch + 14, dx:dx + WO],
                    start=(dx == 0), stop=(dx == 2))
            # reciprocal on ACT, writing bf16 (tolerance 2e-2 >> ACT err)
            nc.scalar.add_instruction(
                mybir.InstActivation(
                    name=nc.get_next_instruction_name(),
                    func=AF.Reciprocal,
                    ins=[nc.scalar.lower_ap(d8p[:]),
                         mybir.ImmediateValue(dtype=F32, value=0.0),
                         mybir.ImmediateValue(dtype=F32, value=1.0),
                         mybir.ImmediateValue(dtype=F32, value=0.0)],
                    outs=[nc.scalar.lower_ap(r8[:, ch * 392:ch * 392 + 392])],
                ))
        return r8

    def keepalive(tag, f):
        # reads e128 frame f so the scheduler can't hoist it before e[f]
        ka = ps_d8.tile([8, 1024], F32, tag="d8", name=f"ka{tag}")
        nc.tensor.matmul(ka[:, 0:512], qm[:, 0:8],
                         e128[:, f * NF:f * NF + 512], start=True, stop=True)

    # ---- T-pass helpers (DVE, 2-frame batched, pair trick) ---------------
    fbase_ap = None

    def pair2(ap_base, t0, t1, width=NF):
        return bass.AP(tensor=ap_base.tensor, offset=ap_base.offset + t0 * width,
                       ap=[list(ap_base.ap[0]), [(t1 - t0) * width, 2], [1, width]])

    def rep2(ap_base, off, width=NF):
        return bass.AP(tensor=ap_base.tensor, offset=ap_base.offset + off,
                       ap=[list(ap_base.ap[0]), [0, 2], [1, width]])

    # ---- W-pass (DVE, 2-frame batched) -----------------------------------
    def w_pass(t0):
        sv = (s128[:, t0 * NF:(t0 + 2) * NF]
              .rearrange("p (t y x) -> p t y x", t=2, y=HP))
        gv = (g128[:, t0 * 840:(t0 + 2) * 840]
              .rearrange("p (t y x) -> p t y x", t=2, y=HP))
        wtmp = sb_n.tile([C, 2 * 840], BF16, tag="wtmp", name=f"wtmp{t0}")
        wv = wtmp[:].rearrange("p (t y x) -> p t y x", t=2, y=HP)
        nc.vector.tensor_add(wv, sv[:, :, :, 0:28], sv[:, :, :, 1:29])
        nc.vector.tensor_add(gv, wv, sv[:, :, :, 2:30])

    # ---- H-pass ----------------------------------------------------------
    def h_pe(t):
        nump = ps_num.tile([C, 784], F32, tag="num", name=f"num{t}")
        gt = g128[:, t * 840:(t + 1) * 840].rearrange("p (y x) -> p y x", y=HP)
        for ch in range(2):
            for dy in range(3):
                nc.tensor.matmul(
                    nump[:, ch * 392:ch * 392 + 392], idm,
                    gt[:, 14 * ch + dy:14 * ch + dy + 14, :],
                    start=(dy == 0), stop=(dy == 2))
        return nump

    def h_dve(t0, numt):
        gv = (g128[:, t0 * 840:(t0 + 2) * 840]
              .rearrange("p (t y x) -> p t y x", t=2, y=HP))
        at = sb_n.tile([C, 2 * NOF], BF16, tag="hA", name=f"hA{t0}")
        av = at[:].rearrange("p (t y x) -> p t y x", t=2, y=HO)
        nv = numt[:].rearrange("p (t y x) -> p t y x", t=2, y=HO)
        nc.vector.tensor_add(av, gv[:, :, 0:28, :], gv[:, :, 1:29, :])
        nc.vector.tensor_add(nv, av, gv[:, :, 2:30, :])

    # ---- tail: r-broadcast, merged, y-proj, evac, out --------------------
    def tail(t, num_ap, r8, merge_engine):
        mt = sb_m.tile([C, NOF], BF16, tag="mt", name=f"mt{t}")
        for ch in range(2):
            rp = ps_rp.tile([C, 392], F32, tag="rp", name=f"rp{t}{ch}")
            nc.tensor.matmul(rp[:],
                             selr[:, t * 128:(t + 1) * 128],
                             r8[:, ch * 392:ch * 392 + 392],
                             start=True, stop=True)
            nc.vector.tensor_mul(mt[:, ch * 392:ch * 392 + 392],
                                 num_ap[:, ch * 392:ch * 392 + 392], rp[:])
        yp = ps_big.tile([C, 1024], F32, tag="big", name=f"yp{t}")
        for ch in range(2):
            nc.tensor.matmul(yp[:, ch * 512:ch * 512 + 392], wt,
                             mt[:, ch * 392:ch * 392 + 392],
                             start=True, stop=True)
        yt = sb_y.tile([C, NOF], BF16, tag="yt", name=f"yt{t}")
        ypv = yp[:].rearrange("p (b k) -> p b k", b=2)[:, :, 0:392]
        ytv = yt[:].rearrange("p (b k) -> p b k", b=2)
        nc.scalar.activation(ytv, ypv, AF.Copy)
        nc.sync.dma_start(out=d_y[:, t], in_=yt[:])

    # ======================= schedule (monotone in t) ====================
    pkB = mk_pack("B")
    pkA = mk_pack("A")
    phase_a(0)
    phase_a(1)
    f_mul(0, "pool", nfr=1)
    phase_a(2)
    phase_a(3)
    f_mul(1, "dve")                   # F1,F2 pair
    pack_all(pkB, 0)                  # frames 0..3
    phase_a(4)
    phase_a(5)
    f_mul(3, "dve")                   # F3,F4 pair
    f_mul(5, "pool", nfr=1)
    pack_all(pkA, 2)                  # frames 2..5

    d8B = den_half("B", pkB, selB)    # tp 0,1
    d8A = den_half("A", pkA, selA)    # tp 2,3

    fv = f128[:]
    # P12 = F1+F2 ; S0|S1 = F0+P12 | F3+P12
    p12 = ptile[:, 0:NF]
    nc.gpsimd.tensor_add(p12, fv[:, NF:2 * NF], fv[:, 2 * NF:3 * NF])
    s01 = bass.AP(tensor=s128[:].tensor, offset=s128[:].offset,
                  ap=[list(s128[:].ap[0]), [NF, 2], [1, NF]])
    nc.vector.tensor_add(s01, pair2(fv, 0, 3), rep2(ptile[:], 0))
    for tt in (0, 1):
        sv1 = (s128[:, tt * NF:(tt + 1) * NF]
               .rearrange("p (y x) -> p y x", y=HP))
        gv1 = (g128[:, tt * 840:(tt + 1) * 840]
               .rearrange("p (y x) -> p y x", y=HP))
        wt1 = sb_n.tile([C, 840], BF16, tag="wt1", name=f"wt1_{tt}")
        wv1 = wt1[:].rearrange("p (y x) -> p y x", y=HP)
        nc.vector.tensor_add(wv1, sv1[:, :, 0:28], sv1[:, :, 1:29])
        nc.vector.tensor_add(gv1, wv1, sv1[:, :, 2:30])
    num01 = sb_n.tile([C, 2 * NOF], BF16, tag="num01", name="num01")
    for tt in (0, 1):
        gv1 = (g128[:, tt * 840:(tt + 1) * 840]
               .rearrange("p (y x) -> p y x", y=HP))
        at1 = sb_n.tile([C, NOF], BF16, tag="hA1", name=f"hA1_{tt}")
        av1 = at1[:].rearrange("p (y x) -> p y x", y=HO)
        nv1 = (num01[:, tt * NOF:(tt + 1) * NOF]
               .rearrange("p (y x) -> p y x", y=HO))
        nc.vector.tensor_add(av1, gv1[:, 0:28, :], gv1[:, 1:29, :])
        nc.vector.tensor_add(nv1, av1, gv1[:, 2:30, :])
    # A-half box chain before the tails: data-ready earlier
    p34 = ptile[:, NF:2 * NF]
    nc.vector.tensor_add(p34, fv[:, 3 * NF:4 * NF], fv[:, 4 * NF:5 * NF])
    s23 = bass.AP(tensor=s128[:].tensor, offset=s128[:].offset + 2 * NF,
                  ap=[list(s128[:].ap[0]), [NF, 2], [1, NF]])
    nc.vector.tensor_add(s23, pair2(fv, 2, 5), rep2(ptile[:], NF))
    for tt in (2, 3):
        sv2 = (s128[:, tt * NF:(tt + 1) * NF]
               .rearrange("p (y x) -> p y x", y=HP))
        gv2 = (g128[:, tt * 840:(tt + 1) * 840]
               .rearrange("p (y x) -> p y x", y=HP))
        wt2 = sb_n.tile([C, 840], BF16, tag="wt1", name=f"wt1_{tt}")
        wv2 = wt2[:].rearrange("p (y x) -> p y x", y=HP)
        nc.vector.tensor_add(wv2, sv2[:, :, 0:28], sv2[:, :, 1:29])
        nc.vector.tensor_add(gv2, wv2, sv2[:, :, 2:30])
    num23 = sb_n.tile([C, 2 * NOF], BF16, tag="num01", name="num23")
    h_dve(2, num23)
    tail(0, num01[:, 0:NOF], d8B, "dve")
    tail(1, num01[:, NOF:2 * NOF], d8B, "dve")
    tail(2, num23[:, 0:NOF], d8A, "dve")
    tail(3, num23[:, NOF:2 * NOF], d8A, "dve")


# ---------------------------------------------------------------------------
# Host side
# ---------------------------------------------------------------------------

def _host_prep(x, q, W_out, b_out):
    B, C_, T, H, W = x.shape
    heads, hs = 2, 64
    cidx = np.arange(C_)
    qfull = (np.asarray(q, np.float32)[cidx % heads, 0, cidx // heads] / hs)
    qm = np.zeros((C_, C_), np.float32)
    for m in range(C_):
        qm[:, m] = np.where(cidx % heads == m % heads, qfull, 0.0)
    idm = np.eye(C_, dtype=np.float32)
    wt = np.ascontiguousarray(np.asarray(W_out, np.float32).T)

    # den sel halves: row (dy,h,dt') = 8dy+4h+dt', col (h',tp) = 4h'+tp
    sel = np.zeros((2, 24, 8), np.float32)
    for half, fbase in ((0, 2), (1, 0)):
        for dy in range(3):
            for h in range(2):
                for dtp in range(4):
                    t = fbase + dtp
                    for tp in range(TOUT):
                        # invalid (half,tp) columns borrow a valid tp's pattern
                        # so their den rows stay positive (no 0*inf NaN)
                        teff = max(tp, 2) if half == 0 else min(tp, 1)
                        if 0 <= t - teff <= 2:
                            sel[half, 8 * dy + 4 * h + dtp, 4 * h + tp] = 1.0
    selr = np.zeros((8, TOUT, C_), np.float32)
    for tp in range(TOUT):
        selr[4 * (cidx % heads) + tp, tp, cidx] = 1.0

    bf = ml_dtypes.bfloat16
    blob = np.zeros((C_, BLOB_W), bf)
    blob[:, 0:128] = qm.astype(bf)
    blob[:, 128:256] = idm.astype(bf)
    blob[:, 256:384] = wt.astype(bf)
    blob[0:24, 384:392] = sel[0].astype(bf)
    blob[0:24, 392:400] = sel[1].astype(bf)
    blob[0:8, 400:912] = selr.reshape(8, TOUT * C_).astype(bf)

    xpad = np.zeros((B, C_, T + 2, H, W), np.float32)
    xpad[:, :, 1:T + 1] = x
    in_maps = []
    for core in range(8):
        b, t0 = core // 4, (core % 4) * 4
        xp = np.ascontiguousarray(
            xpad[b, :, t0:t0 + TIN].reshape(C_, TIN, H * W)).astype(bf)
        in_maps.append({"xp": xp, "blob": blob})
    return in_maps


_NC_CACHE = {}


def _get_nc(reps=1):
    if reps not in _NC_CACHE:
        _NC_CACHE[reps] = _build_nc(reps=reps)
    return _NC_CACHE[reps]


def kernel(x, q, W_out, b_out):
    x = np.asarray(x, np.float32)
    in_maps = _host_prep(x, q, W_out, b_out)
    nc = _get_nc()
    res = bass_utils.run_bass_kernel_spmd(nc, in_maps, list(range(8)))
    y = np.zeros((2, 128, 16, 28, 28), np.float32)
    for core in range(8):
        b, t0 = core // 4, (core % 4) * 4
        y[b, :, t0:t0 + TOUT] = np.asarray(
            res.results[core]["y"]).astype(np.float32).reshape(C, TOUT, HO, WO)
    y += np.asarray(b_out, np.float32)[None, :, None, None, None]
    return y
